# revision 2
# baseline (speedup 1.0000x reference)
"""Trainium2 Bass kernel for nn_Cross_Scale_Mamba_Block.

Sharding: data-parallel over batch — 1 sample per NeuronCore, 8 cores.
Per core the three VSS (SS2D selective-scan) chunks run sequentially.
The selective scan uses the DVE tensor_tensor_scan instruction
(h[t] = a[t]*h[t-1] + b[t] along the free dim at line rate) with
state rows = (k-pair, d) on partitions, one scan slice per state n.
B/C broadcasts across the 64 d-partitions go through PE selector
matmuls into PSUM.  Time-reversed directions k2/k3 use negative-step
access patterns on the scan operands only; all other tensors stay in
normal (unflipped) time order.
"""
import sys

sys.path.insert(0, "/opt/trn_rl_repo")

import contextlib

import numpy as np

import concourse.bass as bass
import concourse.bacc as bacc
import concourse.tile as tile
import concourse.mybir as mybir

F32 = mybir.dt.float32
BF16 = mybir.dt.bfloat16
AX = mybir.AluOpType
ACTF = mybir.ActivationFunctionType

B, DIM, HH, WW = 8, 128, 64, 64
L = HH * WW                      # 4096
C, DI, N, RR, K = 32, 64, 16, 2, 4
EPS = 1e-5
LC = 2048                        # scan L-chunk
NCH = L // LC

LAST_EXEC_NS = None
_CACHE = {}


def _flip(ap):
    """Reverse the last (free) dim of an AP view."""
    a = ap.copy()
    apl = [list(x) for x in a.ap]
    step, cnt = apl[-1]
    assert step > 0
    return bass.AP(a.tensor, a.offset + step * (cnt - 1),
                   apl[:-1] + [[-step, cnt]])


# ---------------------------------------------------------------- weights

def build_weights(inputs):
    f = np.float32
    w = {}
    ln_w = inputs["ln_w"].astype(f)
    ln_b = inputs["ln_b"].astype(f)
    in_proj_w = inputs["in_proj_w"].astype(f)           # [128, 32]
    w_in = in_proj_w * ln_w[None, :]                    # gamma fold
    w["w_inT"] = np.ascontiguousarray(w_in.T)           # [32, 128]
    beta_vec = in_proj_w @ ln_b                         # [128]
    conv_w = inputs["conv_w"].astype(f)                 # [64,1,3,3]
    conv_b = inputs["conv_b"].astype(f)
    cb2 = conv_b + beta_vec[:DI] * conv_w.reshape(DI, 9).sum(1)
    w["convW"] = conv_w.reshape(DI, 9)
    w["convB"] = cb2.reshape(DI, 1)
    w["z_bias"] = beta_vec[DI:].reshape(DI, 1)

    xp = inputs["x_proj_w"].astype(f)                   # [4, 34, 64]
    dtw = inputs["dt_proj_w"].astype(f)                 # [4, 64, 2]
    for k in range(K):
        bc = np.zeros((64, 48), f)
        bc[:, 0:16] = xp[k][2:18].T                      # B
        bc[:, 32:48] = xp[k][18:34].T                    # C
        w[f"xprojBCT{k}"] = bc
        w[f"wdtT{k}"] = np.ascontiguousarray((dtw[k] @ xp[k][:2]).T)
    dtb = inputs["dt_proj_b"].astype(f)                 # [4, 64]
    A = -np.exp(inputs["A_log"].astype(f))              # [4, 64, 16]
    D = inputs["D"].astype(f)                           # [4, 64]
    out_norm_w = inputs["out_norm_w"].astype(f)
    out_proj_w = inputs["out_proj_w"].astype(f)         # [32, 64]
    wout = out_proj_w * out_norm_w[None, :]             # gamma fold
    w["w_outT"] = np.ascontiguousarray(wout.T)          # [64, 32]

    for t, (klo, khi) in enumerate(((0, 1), (2, 3))):
        w[f"dtb_col{t}"] = np.concatenate(
            [dtb[klo], dtb[khi]]).reshape(128, 1)
        w[f"A_cols{t}"] = np.concatenate([A[klo], A[khi]], axis=0)
        dl = np.zeros((64, 128), f)
        dl[np.arange(64), np.arange(64)] = D[klo]
        dh = np.zeros((64, 128), f)
        dh[np.arange(64), 64 + np.arange(64)] = D[khi]
        w[f"DdiagL{t}"] = dl
        w[f"DdiagH{t}"] = dh

    for i in range(3):
        for nm in ("mh", "mw"):
            w[f"{nm}W{i}"] = inputs[f"dw{i+1}_{nm}_w"].astype(f).reshape(C, 7)
            w[f"{nm}B{i}"] = inputs[f"dw{i+1}_{nm}_b"].astype(f).reshape(C, 1)
        w[f"ccW{i}"] = inputs[f"dw{i+1}_c_w"].astype(f).reshape(C, 9)
        w[f"ccB{i}"] = inputs[f"dw{i+1}_c_b"].astype(f).reshape(C, 1)

    bn_g = inputs["bn_g"].astype(f)
    bn_b = inputs["bn_b"].astype(f)
    bn_m = inputs["bn_m"].astype(f)
    bn_v = inputs["bn_v"].astype(f)
    s = bn_g / np.sqrt(bn_v + EPS)
    w["bn_s"] = s.reshape(128, 1)
    w["bn_bias"] = (bn_b - bn_m * s).reshape(128, 1)

    w["ident"] = np.eye(128, dtype=f)
    w["eps_col"] = np.full((128, 1), EPS, f)

    w["identB"] = w["ident"].copy()
    for n in range(N):
        seln = np.zeros((64, 128), f)
        seln[n, :64] = 1.0
        seln[32 + n, 64:] = 1.0
        w[f"seln{n}"] = seln
    import ml_dtypes
    for name, (_shape, dt_) in WSPEC.items():
        if dt_ == "b":
            w[name] = np.asarray(w[name]).astype(ml_dtypes.bfloat16)
    return w


WSPEC = {
    "w_inT": ((32, 128), "f"), "convW": ((DI, 9), "f"),
    "convB": ((DI, 1), "f"), "z_bias": ((DI, 1), "f"),
    "w_outT": ((64, 32), "f"), "ident": ((128, 128), "f"),
    "identB": ((128, 128), "b"),
    "bn_s": ((128, 1), "f"), "bn_bias": ((128, 1), "f"),
    "eps_col": ((128, 1), "f"),
}
for _n in range(N):
    WSPEC[f"seln{_n}"] = ((64, 128), "b")
for _k in range(K):
    WSPEC[f"xprojBCT{_k}"] = ((64, 48), "b")
    WSPEC[f"wdtT{_k}"] = ((64, 64), "b")
for _t in range(2):
    WSPEC[f"dtb_col{_t}"] = ((128, 1), "f")
    WSPEC[f"A_cols{_t}"] = ((128, 16), "f")
    WSPEC[f"DdiagL{_t}"] = ((64, 128), "b")
    WSPEC[f"DdiagH{_t}"] = ((64, 128), "b")
for _i in range(3):
    for _nm in ("mh", "mw"):
        WSPEC[f"{_nm}W{_i}"] = ((C, 7), "f")
        WSPEC[f"{_nm}B{_i}"] = ((C, 1), "f")
    WSPEC[f"ccW{_i}"] = ((C, 9), "f")
    WSPEC[f"ccB{_i}"] = ((C, 1), "f")


# ---------------------------------------------------------------- program

def tap_conv(nc, x3, acc3, wcol, bcol, offs):
    """Depthwise conv via shifted tap accumulation, SAME zero padding.
    x3/acc3: [P, d1, d2] views.  offs: [(tap_idx, o1, o2)]; first = center."""
    d1, d2 = x3.shape[1], x3.shape[2]
    first = True
    for (j, o1, o2) in offs:
        lo1, hi1 = max(0, -o1), d1 - max(0, o1)
        lo2, hi2 = max(0, -o2), d2 - max(0, o2)
        src = x3[:, lo1 + o1:hi1 + o1, lo2 + o2:hi2 + o2]
        dst = acc3[:, lo1:hi1, lo2:hi2]
        if first:
            assert o1 == 0 and o2 == 0
            nc.vector.tensor_scalar(acc3[:, :, :], x3[:, :, :],
                                    wcol[:, j:j + 1], bcol[:, 0:1],
                                    AX.mult, AX.add)
            first = False
        else:
            nc.vector.scalar_tensor_tensor(dst, src, wcol[:, j:j + 1],
                                           dst, AX.mult, AX.add)


def build_vss_chunk(nc, tc, W, x_sb, res, ci, dil, main, scr, scanp,
                    scan1, psum, psy):
    r0 = 32 * ci
    ident = W["ident"]

    # ---- Stage A: axial depthwise convs + skip -> t_i [32, L]
    xi0 = scr.tile([C, L], F32, tag="slot4")
    nc.vector.tensor_copy(xi0[:], x_sb[r0:r0 + 32, :])
    xi = xi0[:]
    xi3 = xi.rearrange("p (a b) -> p a b", a=HH)
    acc1 = scr.tile([C, L], F32, tag="slot1")
    acc2 = scr.tile([C, L], F32, tag="slot2")
    t_i = main.tile([C, L], F32, tag="t_i")
    a13 = acc1[:].rearrange("p (a b) -> p a b", a=HH)
    a23 = acc2[:].rearrange("p (a b) -> p a b", a=HH)
    offs = [(3, 0, 0)] + [(j, 0, (j - 3) * dil) for j in range(7) if j != 3]
    tap_conv(nc, xi3, a13, W[f"mwW{ci}"], W[f"mwB{ci}"], offs)
    offs = [(3, 0, 0)] + [(j, (j - 3) * dil, 0) for j in range(7) if j != 3]
    tap_conv(nc, a13, a23, W[f"mhW{ci}"], W[f"mhB{ci}"], offs)
    offs = [(4, 0, 0)] + [(3 * (dh + 1) + (dw + 1), dh * dil, dw * dil)
                          for dh in (-1, 0, 1) for dw in (-1, 0, 1)
                          if not (dh == 0 and dw == 0)]
    ti3 = t_i[:].rearrange("p (a b) -> p a b", a=HH)
    tap_conv(nc, a23, ti3, W[f"ccW{ci}"], W[f"ccB{ci}"], offs)
    nc.vector.tensor_tensor(t_i[:], t_i[:], xi, AX.add)

    # ---- Stage B: LayerNorm over c (t-partition layout)
    tT = scr.tile([128, 1024], BF16, tag="tT")    # [t128, (blk32, c32)]
    for half in range(2):
        ps = psum.tile([128, 512], F32, tag="ps")
        for b2 in range(16):
            blk = half * 16 + b2
            nc.tensor.transpose(ps[:, 32 * b2:32 * b2 + 32],
                                t_i[:, 128 * blk:128 * blk + 128],
                                ident[0:32, 0:32])
        nc.scalar.activation(tT[:, 512 * half:512 * half + 512], ps[:],
                             ACTF.Copy)
    t3 = tT[:].rearrange("p (a b) -> p a b", a=32)
    mu = scr.tile([128, 32], F32, tag="mu")
    nc.vector.tensor_reduce(mu[:, :, None], t3, mybir.AxisListType.X, AX.add)
    nc.vector.tensor_scalar_mul(mu[:], mu[:], -1.0 / 32)
    nc.vector.tensor_tensor(t3, t3,
                            mu[:, :, None].broadcast_to((128, 32, 32)),
                            AX.add)
    sq = scr.tile([128, 1024], F32, tag="sq")
    sq3 = sq[:].rearrange("p (a b) -> p a b", a=32)
    nc.vector.tensor_tensor(sq3, t3, t3, AX.mult)
    var = scr.tile([128, 32], F32, tag="var")
    nc.vector.tensor_reduce(var[:, :, None], sq3, mybir.AxisListType.X,
                            AX.add)
    nc.scalar.activation(var[:], var[:], ACTF.Ln,
                         bias=W["eps_col"][:, 0:1], scale=1.0 / 32)
    nc.scalar.activation(var[:], var[:], ACTF.Exp, scale=-0.5)
    nc.vector.tensor_tensor(t3, t3,
                            var[:, :, None].broadcast_to((128, 32, 32)),
                            AX.mult)
    # transpose back -> ln [32, L]
    ln = scr.tile([C, L], F32, tag="slot2")
    for g in range(8):
        ps = psy.tile([128, 512], BF16, tag="ypsum")
        for b2 in range(4):
            blk = g * 4 + b2
            nc.tensor.transpose(ps[0:32, 128 * b2:128 * b2 + 128],
                                tT[:, 32 * blk:32 * blk + 32],
                                W["identB"][:, :])
        nc.scalar.activation(ln[:, 512 * g:512 * g + 512], ps[0:32, :],
                             ACTF.Copy)

    # ---- Stage C: in_proj -> xc [64, L], sz = silu(z + zb) [64, L] bf16
    xc = scr.tile([DI, L], F32, tag="slot3")
    sz = main.tile([DI, L], BF16, tag="sz")
    for ch in range(8):
        ps = psum.tile([128, 512], F32, tag="ps")
        nc.tensor.matmul(ps[:], W["w_inT"][:], ln[:, 512 * ch:512 * ch + 512])
        nc.scalar.activation(xc[:, 512 * ch:512 * ch + 512], ps[0:64, :],
                             ACTF.Copy)
        sgs = scr.tile([64, 512], F32, tag="tT")
        nc.scalar.activation(sgs[:], ps[64:128, :], ACTF.Sigmoid,
                             bias=W["z_bias"][:, 0:1])
        nc.vector.scalar_tensor_tensor(sz[:, 512 * ch:512 * ch + 512],
                                       ps[64:128, :], W["z_bias"][:, 0:1],
                                       sgs[:], AX.add, AX.mult)

    # ---- Stage D: depthwise 3x3 (dil 1) + silu -> u tiles (bf16)
    u_t = main.tile([128, L], BF16, tag="u_t")   # rows: hw | wh
    u_hw = main.tile([DI, L], BF16, tag="u_hw")  # base-0 copy for PE rhs
    u_wh = main.tile([DI, L], BF16, tag="u_wh")
    ud = scr.tile([DI, L], F32, tag="slot4")
    xc3 = xc[:].rearrange("p (a b) -> p a b", a=HH)
    ud3 = ud[:].rearrange("p (a b) -> p a b", a=HH)
    offs = [(4, 0, 0)] + [(3 * (dh + 1) + (dw + 1), dh, dw)
                          for dh in (-1, 0, 1) for dw in (-1, 0, 1)
                          if not (dh == 0 and dw == 0)]
    tap_conv(nc, xc3, ud3, W["convW"], W["convB"], offs)
    sg2 = scr.tile([DI, L], F32, tag="slot3")
    nc.scalar.activation(sg2[:], ud[:], ACTF.Sigmoid)
    nc.vector.tensor_tensor(u_hw[:], ud[:], sg2[:], AX.mult)
    nc.vector.tensor_copy(u_t[0:64, :], u_hw[:])
    u3 = u_hw[:].rearrange("p (a b) -> p a b", a=HH)
    nc.vector.tensor_copy(u_wh[:].rearrange("p (a b) -> p a b", a=WW),
                          u3.transpose([0, 2, 1]))
    nc.vector.tensor_copy(u_t[64:128, :], u_wh[:])

    # ---- Stages E-F per k-pair tile
    oy = main.tile([DI, L], BF16, tag="oy")      # hw half (k0+k2)
    oyW = main.tile([DI, L], BF16, tag="oyW")    # wh half (k1+k3)
    for t in range(2):
        dt_t = scr.tile([128, L], F32, tag="slot3")
        w_t = scr.tile([128, L], BF16, tag="slot4h")
        # row layout in each: klo 0-15, khi 32-47
        bB_t = scr.tile([64, L], BF16, tag="bB_t")
        bC_t = scr.tile([64, L], BF16, tag="bC_t")
        nc.gpsimd.memset(bB_t[:], 0.0)
        nc.gpsimd.memset(bC_t[:], 0.0)
        for kl in range(2):
            k = 2 * t + kl
            uk = (u_hw if kl == 0 else u_wh)[:, :]
            xbc = scr.tile([48, L], F32, tag="slot1")
            for ch in range(8):
                ps = psum.tile([128, 512], F32, tag="ps")
                nc.tensor.matmul(ps[0:48, :], W[f"xprojBCT{k}"][:],
                                 uk[:, 512 * ch:512 * ch + 512])
                nc.scalar.activation(xbc[:, 512 * ch:512 * ch + 512],
                                     ps[0:48, :], ACTF.Copy)
            nc.gpsimd.tensor_copy(bB_t[32 * kl:32 * kl + 16, :],
                                  xbc[0:16, :])
            nc.gpsimd.tensor_copy(bC_t[32 * kl:32 * kl + 16, :],
                                  xbc[32:48, :])
            for ch in range(8):
                ps = psum.tile([128, 512], F32, tag="ps")
                nc.tensor.matmul(ps[0:64, :], W[f"wdtT{k}"][:],
                                 uk[:, 512 * ch:512 * ch + 512])
                sps = scr.tile([64, 512], F32, tag="tT")
                nc.scalar.activation(
                    sps[:], ps[0:64, :], ACTF.Exp,
                    bias=W[f"dtb_col{t}"][64 * kl:64 * kl + 64, 0:1])
                nc.scalar.activation(
                    dt_t[64 * kl:64 * kl + 64, 512 * ch:512 * ch + 512],
                    sps[:], ACTF.Ln, bias=1.0)


        nc.vector.tensor_tensor(w_t[:], dt_t[:], u_t[:], AX.mult)
        flip = (t == 1)
        carry = scr.tile([128, N], F32, tag="carry")
        lcs = list(range(NCH)) if not flip else list(reversed(range(NCH)))
        for ilc, lc in enumerate(lcs):
            sl = slice(LC * lc, LC * lc + LC)
            yps = [psy.tile([128, 512], F32, tag="ypsum", name=f"yp{j2}")
                   for j2 in range(LC // 512)]
            for n in range(N):
                a_n = scanp.tile([128, LC], BF16, tag="a_n")
                nc.scalar.activation(a_n[:], dt_t[:, sl], ACTF.Exp,
                                     scale=W[f"A_cols{t}"][:, n:n + 1])
                b_n = scanp.tile([128, LC], BF16, tag="b_n")
                for j2 in range(LC // 512):
                    s3 = slice(LC * lc + 512 * j2, LC * lc + 512 * j2 + 512)
                    psb = psum.tile([128, 512], F32, tag="ps")
                    nc.tensor.matmul(psb[:], W[f"seln{n}"][:], bB_t[:, s3])
                    nc.vector.tensor_tensor(
                        b_n[:, 512 * j2:512 * j2 + 512],
                        w_t[:, s3], psb[:], AX.mult)
                h_n = scan1.tile([128, LC], BF16, tag="h_n")
                init = 0.0 if ilc == 0 else carry[:, n:n + 1]
                if not flip:
                    nc.vector.tensor_tensor_scan(h_n[:], a_n[:], b_n[:],
                                                 init, AX.mult, AX.add)
                    nc.vector.tensor_copy(carry[:, n:n + 1],
                                          h_n[:, LC - 1:LC])
                else:
                    nc.vector.tensor_tensor_scan(_flip(h_n[:]), _flip(a_n[:]),
                                                 _flip(b_n[:]), init,
                                                 AX.mult, AX.add)
                    nc.vector.tensor_copy(carry[:, n:n + 1], h_n[:, 0:1])
                hc_n = scanp.tile([128, LC], BF16, tag="b_n")
                for j2 in range(LC // 512):
                    j3 = slice(512 * j2, 512 * j2 + 512)
                    s3 = slice(LC * lc + 512 * j2, LC * lc + 512 * j2 + 512)
                    psc = psum.tile([128, 512], F32, tag="ps")
                    nc.tensor.matmul(psc[:], W[f"seln{n}"][:], bC_t[:, s3])
                    nc.vector.tensor_tensor(hc_n[:, j3], h_n[:, j3], psc[:],
                                            AX.mult)
                for j in range(LC // 512):
                    nc.tensor.matmul(yps[j][:], W["identB"][:, :],
                                     hc_n[:, 512 * j:512 * j + 512],
                                     start=(n == 0), stop=False)
            for j in range(LC // 512):
                s2 = slice(LC * lc + 512 * j, LC * lc + 512 * j + 512)
                nc.tensor.matmul(yps[j][:], W[f"DdiagL{t}"][:], u_hw[:, s2],
                                 start=False, stop=False)
                nc.tensor.matmul(yps[j][:], W[f"DdiagH{t}"][:], u_wh[:, s2],
                                 start=False, stop=True)
                if t == 0:
                    nc.vector.tensor_copy(oy[:, s2], yps[j][0:64, :])
                    nc.vector.tensor_copy(oyW[:, s2], yps[j][64:128, :])
                else:
                    nc.vector.tensor_tensor(oy[:, s2], oy[:, s2],
                                            yps[j][0:64, :], AX.add)
                    nc.vector.tensor_tensor(oyW[:, s2], oyW[:, s2],
                                            yps[j][64:128, :], AX.add)

    # ---- Stage G: combine directions -> y_d [64, L]
    y_d = scr.tile([DI, L], F32, tag="slot1")
    oy_wh = oyW[:].rearrange("p (a b) -> p a b", a=WW)
    y3 = y_d[:].rearrange("p (a b) -> p a b", a=HH)
    nc.vector.tensor_tensor(
        y3, oy[:].rearrange("p (a b) -> p a b", a=HH),
        oy_wh.transpose([0, 2, 1]), AX.add)

    # ---- Stage H: out-LN over d, gate with silu(z), out_proj, residual
    yT = scr.tile([128, 2048], F32, tag="yT")    # [t128, (blk32, d64)]
    for g in range(4):
        ps = psum.tile([128, 512], F32, tag="ps")
        for b2 in range(8):
            blk = g * 8 + b2
            nc.tensor.transpose(ps[:, 64 * b2:64 * b2 + 64],
                                y_d[:, 128 * blk:128 * blk + 128],
                                ident[0:64, 0:64])
        nc.scalar.activation(yT[:, 512 * g:512 * g + 512], ps[:], ACTF.Copy)
    y3T = yT[:].rearrange("p (a b) -> p a b", a=32)
    mu2 = scr.tile([128, 32], F32, tag="mu")
    nc.vector.tensor_reduce(mu2[:, :, None], y3T, mybir.AxisListType.X,
                            AX.add)
    nc.vector.tensor_scalar_mul(mu2[:], mu2[:], -1.0 / 64)
    nc.vector.tensor_tensor(y3T, y3T,
                            mu2[:, :, None].broadcast_to((128, 32, 64)),
                            AX.add)
    var2 = scr.tile([128, 32], F32, tag="var")
    for hf in range(2):
        sq2 = scr.tile([128, 1024], F32, tag="sq")
        sq23 = sq2[:].rearrange("p (a b) -> p a b", a=16)
        y3Th = yT[:, 1024 * hf:1024 * hf + 1024].rearrange(
            "p (a b) -> p a b", a=16)
        nc.vector.tensor_tensor(sq23, y3Th, y3Th, AX.mult)
        nc.vector.tensor_reduce(var2[:, 16 * hf:16 * hf + 16, None], sq23,
                                mybir.AxisListType.X, AX.add)
    nc.scalar.activation(var2[:], var2[:], ACTF.Ln,
                         bias=W["eps_col"][:, 0:1], scale=1.0 / 64)
    nc.scalar.activation(var2[:], var2[:], ACTF.Exp, scale=-0.5)
    nc.vector.tensor_tensor(y3T, y3T,
                            var2[:, :, None].broadcast_to((128, 32, 64)),
                            AX.mult)
    # gate with silu(z) (transposed via PE)
    for g in range(4):
        ps = psy.tile([128, 512], BF16, tag="ypsum")
        for b2 in range(8):
            blk = g * 8 + b2
            nc.tensor.transpose(ps[:, 64 * b2:64 * b2 + 64],
                                sz[:, 128 * blk:128 * blk + 128],
                                W["identB"][0:64, 0:64])
        nc.vector.tensor_tensor(yT[:, 512 * g:512 * g + 512],
                                yT[:, 512 * g:512 * g + 512], ps[:], AX.mult)
    # transpose back -> gy [64, L]
    gy = scr.tile([DI, L], F32, tag="slot2")
    for g in range(8):
        ps = psum.tile([128, 512], F32, tag="ps")
        for b2 in range(4):
            blk = g * 4 + b2
            nc.tensor.transpose(ps[0:64, 128 * b2:128 * b2 + 128],
                                yT[:, 64 * blk:64 * blk + 64], ident[:, :])
        nc.scalar.activation(gy[:, 512 * g:512 * g + 512], ps[0:64, :],
                             ACTF.Copy)
    # out_proj + residual -> res rows
    for ch in range(8):
        ps = psum.tile([128, 512], F32, tag="ps")
        nc.tensor.matmul(ps[0:32, :], W["w_outT"][:],
                         gy[:, 512 * ch:512 * ch + 512])
        nc.vector.tensor_tensor(res[r0:r0 + 32, 512 * ch:512 * ch + 512],
                                t_i[:, 512 * ch:512 * ch + 512],
                                ps[0:32, :], AX.add)


def build_program():
    nc = bacc.Bacc("TRN2", debug=False, num_devices=8)
    xs_d = nc.dram_tensor("xs", [128, L], F32, kind="ExternalInput")
    wts = {name: nc.dram_tensor(name, list(shape),
                                F32 if dt_ == "f" else BF16,
                                kind="ExternalInput")
           for name, (shape, dt_) in WSPEC.items()}
    out_d = nc.dram_tensor("out", [128, L], F32, kind="ExternalOutput")

    with tile.TileContext(nc) as tc:
        with contextlib.ExitStack() as ctx:
            cpool = ctx.enter_context(tc.tile_pool(name="consts", bufs=1))
            main = ctx.enter_context(tc.tile_pool(name="main", bufs=1))
            scr = ctx.enter_context(tc.tile_pool(name="scr", bufs=1))
            scanp = ctx.enter_context(tc.tile_pool(name="scanp", bufs=1))
            scan1 = ctx.enter_context(tc.tile_pool(name="scan1", bufs=1))
            psum = ctx.enter_context(
                tc.tile_pool(name="ps", bufs=4, space="PSUM"))
            psy = ctx.enter_context(
                tc.tile_pool(name="psy", bufs=4, space="PSUM"))

            W = {}
            for name, (shape, dt_) in WSPEC.items():
                t = cpool.tile(list(shape), F32 if dt_ == "f" else BF16,
                               tag=f"w_{name}")
                nc.sync.dma_start(t[:], wts[name][:])
                W[name] = t

            x_sb = main.tile([128, L], F32, tag="x_sb")
            nc.sync.dma_start(x_sb[:], xs_d[:])
            res = x_sb    # vss chunk results overwrite consumed x rows

            for ci in range(3):
                build_vss_chunk(nc, tc, W, x_sb, res, ci, ci + 1,
                                main, scr, scanp, scan1, psum, psy)

            nc.scalar.activation(x_sb[:], res[:], ACTF.Relu,
                                 bias=W["bn_bias"][:, 0:1],
                                 scale=W["bn_s"][:, 0:1])
            nc.sync.dma_start(out_d[:], x_sb[:])

    nc.compile()
    return nc


def get_program():
    if "nc" not in _CACHE:
        _CACHE["nc"] = build_program()
    return _CACHE["nc"]


def kernel(**inputs):
    import os
    from concourse.bass_utils import run_bass_kernel_spmd
    global LAST_EXEC_NS
    nc = get_program()
    w = build_weights(inputs)
    x = np.asarray(inputs["x"], np.float32)          # [8, 128, 64, 64]
    in_maps = []
    for core in range(8):
        m = dict(w)
        m["xs"] = np.ascontiguousarray(x[core].reshape(128, L))
        in_maps.append(m)
    kw = {}
    if os.environ.get("BASS_PROF_DIR"):
        kw = dict(tmpdir=os.environ["BASS_PROF_DIR"], trace=True)
    res = run_bass_kernel_spmd(nc, in_maps, list(range(8)), **kw)
    LAST_EXEC_NS = res.exec_time_ns
    out = np.stack([res.results[i]["out"] for i in range(8)])
    return out.reshape(8, 128, HH, WW).astype(np.float32)



# revision 10
# speedup vs baseline: 1.4814x; 1.4814x over previous
"""Trainium2 Bass kernel for nn_Cross_Scale_Mamba_Block.

Sharding: data-parallel over batch — 1 sample per NeuronCore, 8 cores.
Per core the three VSS (SS2D selective-scan) chunks run sequentially.

v2 layout notes:
 - dt/w/u/scan tensors are bf16 end-to-end so DVE tensor_tensor runs in
   2x_1P mode and scalar activations in 2x.
 - dt-proj and x-proj matmuls use block-diagonal stationaries so one
   matmul covers both directions of a k-pair (rhs = u_t, 128 rows).
 - softplus/silu use the native ACT table functions.
 - B/C broadcasts: PE one-hot matmuls (selnB/selnC) from a packed
   bbc[64,L] tile -> PSUM, scalar copies PSUM->SBUF bf16 staging, DVE
   multiplies at 2x; the C-side product runs on GpSimd to offload DVE.
 - 3x3 depthwise conv of SS2D (stage D) runs on the PE as 9 shifted
   diagonal matmuls over a zero-padded bf16 image.
 - time-reversed directions use negative-step APs on the scan operands.
"""
import sys

sys.path.insert(0, "/opt/trn_rl_repo")

import contextlib

import numpy as np

import concourse.bass as bass
import concourse.bacc as bacc
import concourse.tile as tile
import concourse.mybir as mybir

F32 = mybir.dt.float32
BF16 = mybir.dt.bfloat16
AX = mybir.AluOpType
ACTF = mybir.ActivationFunctionType

B, DIM, HH, WW = 8, 128, 64, 64
L = HH * WW                      # 4096
C, DI, N, RR, K = 32, 64, 16, 2, 4
EPS = 1e-5
LC = 2048                        # scan L-chunk
NCH = L // LC
PW = WW + 2                      # padded width for 3x3 conv image

LAST_EXEC_NS = None
_CACHE = {}


def _flip(ap):
    """Reverse the last (free) dim of an AP view."""
    a = ap.copy()
    apl = [list(x) for x in a.ap]
    step, cnt = apl[-1]
    assert step > 0
    return bass.AP(a.tensor, a.offset + step * (cnt - 1),
                   apl[:-1] + [[-step, cnt]])


# ---------------------------------------------------------------- weights

def build_weights(inputs):
    f = np.float32
    w = {}
    ln_w = inputs["ln_w"].astype(f)
    ln_b = inputs["ln_b"].astype(f)
    in_proj_w = inputs["in_proj_w"].astype(f)           # [128, 32]
    w_in = in_proj_w * ln_w[None, :]                    # gamma fold
    w["w_inT"] = np.ascontiguousarray(w_in.T)           # [32, 128]
    beta_vec = in_proj_w @ ln_b                         # [128]
    conv_w = inputs["conv_w"].astype(f)                 # [64,1,3,3]
    conv_b = inputs["conv_b"].astype(f)
    cb2 = conv_b + beta_vec[:DI] * conv_w.reshape(DI, 9).sum(1)
    cd = np.zeros((DI, 9 * DI), f)                      # 9 diag blocks
    w9 = conv_w.reshape(DI, 9)
    for j in range(9):
        cd[np.arange(DI), j * DI + np.arange(DI)] = w9[:, j]
    w["convDiag"] = cd
    w["convB"] = cb2.reshape(DI, 1)
    w["z_bias"] = beta_vec[DI:].reshape(DI, 1)

    xp = inputs["x_proj_w"].astype(f)                   # [4, 34, 64]
    dtw = inputs["dt_proj_w"].astype(f)                 # [4, 64, 2]
    for t, (klo, khi) in enumerate(((0, 1), (2, 3))):
        # x-proj block-diagonal: rhs = u_t (hw rows 0-63, wh rows 64-127)
        # out rows: 0-15 B(klo), 16-31 C(klo), 32-47 B(khi), 48-63 C(khi)
        xb = np.zeros((128, 64), f)
        xb[0:64, 0:16] = xp[klo][2:18].T
        xb[0:64, 16:32] = xp[klo][18:34].T
        xb[64:128, 32:48] = xp[khi][2:18].T
        xb[64:128, 48:64] = xp[khi][18:34].T
        w[f"xprojBD{t}"] = xb
        # dt-proj block-diagonal [128 -> 128]
        w1 = dtw[klo] @ xp[klo][:2]                     # [64(d), 64(k)]
        w2 = dtw[khi] @ xp[khi][:2]
        db = np.zeros((128, 128), f)
        db[0:64, 0:64] = w1.T
        db[64:128, 64:128] = w2.T
        w[f"wdtBD{t}"] = db
    dtb = inputs["dt_proj_b"].astype(f)                 # [4, 64]
    A = -np.exp(inputs["A_log"].astype(f))              # [4, 64, 16]
    D = inputs["D"].astype(f)                           # [4, 64]
    out_norm_w = inputs["out_norm_w"].astype(f)
    out_proj_w = inputs["out_proj_w"].astype(f)         # [32, 64]
    wout = out_proj_w * out_norm_w[None, :]             # gamma fold
    w["w_outT"] = np.ascontiguousarray(wout.T)          # [64, 32]

    for t, (klo, khi) in enumerate(((0, 1), (2, 3))):
        w[f"dtb_col{t}"] = np.concatenate(
            [dtb[klo], dtb[khi]]).reshape(128, 1)
        w[f"A_cols{t}"] = np.concatenate([A[klo], A[khi]], axis=0)
        dl = np.zeros((64, 128), f)
        dl[np.arange(64), np.arange(64)] = D[klo]
        dh = np.zeros((64, 128), f)
        dh[np.arange(64), 64 + np.arange(64)] = D[khi]
        w[f"DdiagL{t}"] = dl
        w[f"DdiagH{t}"] = dh

    for i in range(3):
        for nm in ("mh", "mw"):
            w[f"{nm}W{i}"] = inputs[f"dw{i+1}_{nm}_w"].astype(f).reshape(C, 7)
            w[f"{nm}B{i}"] = inputs[f"dw{i+1}_{nm}_b"].astype(f).reshape(C, 1)
        w[f"ccW{i}"] = inputs[f"dw{i+1}_c_w"].astype(f).reshape(C, 9)
        w[f"ccB{i}"] = inputs[f"dw{i+1}_c_b"].astype(f).reshape(C, 1)

    bn_g = inputs["bn_g"].astype(f)
    bn_b = inputs["bn_b"].astype(f)
    bn_m = inputs["bn_m"].astype(f)
    bn_v = inputs["bn_v"].astype(f)
    s = bn_g / np.sqrt(bn_v + EPS)
    w["bn_s"] = s.reshape(128, 1)
    w["bn_bias"] = (bn_b - bn_m * s).reshape(128, 1)

    w["ident"] = np.eye(128, dtype=f)
    w["eps_col"] = np.full((128, 1), EPS, f)

    w["identB"] = w["ident"].copy()
    for n in range(N):
        # bbc rows: 0-15 B(klo), 16-31 C(klo), 32-47 B(khi), 48-63 C(khi)
        sb = np.zeros((64, 128), f)
        sb[n, 0:64] = 1.0
        sb[32 + n, 64:128] = 1.0
        w[f"selnB{n}"] = sb
        sc = np.zeros((64, 128), f)
        sc[16 + n, 0:64] = 1.0
        sc[48 + n, 64:128] = 1.0
        w[f"selnC{n}"] = sc
    import ml_dtypes
    for name, (_shape, dt_) in WSPEC.items():
        if dt_ == "b":
            w[name] = np.asarray(w[name]).astype(ml_dtypes.bfloat16)
    return w


WSPEC = {
    "w_inT": ((32, 128), "b"),
    "convDiag": ((DI, 9 * DI), "b"),
    "convB": ((DI, 1), "f"), "z_bias": ((DI, 1), "f"),
    "w_outT": ((64, 32), "b"), "ident": ((128, 128), "f"),
    "identB": ((128, 128), "b"),
    "bn_s": ((128, 1), "f"), "bn_bias": ((128, 1), "f"),
    "eps_col": ((128, 1), "f"),
}
for _n in range(N):
    WSPEC[f"selnB{_n}"] = ((64, 128), "b")
    WSPEC[f"selnC{_n}"] = ((64, 128), "b")
for _t in range(2):
    WSPEC[f"xprojBD{_t}"] = ((128, 64), "b")
    WSPEC[f"wdtBD{_t}"] = ((128, 128), "b")
    WSPEC[f"dtb_col{_t}"] = ((128, 1), "f")
    WSPEC[f"A_cols{_t}"] = ((128, 16), "f")
    WSPEC[f"DdiagL{_t}"] = ((64, 128), "b")
    WSPEC[f"DdiagH{_t}"] = ((64, 128), "b")
for _i in range(3):
    for _nm in ("mh", "mw"):
        WSPEC[f"{_nm}W{_i}"] = ((C, 7), "f")
        WSPEC[f"{_nm}B{_i}"] = ((C, 1), "f")
    WSPEC[f"ccW{_i}"] = ((C, 9), "f")
    WSPEC[f"ccB{_i}"] = ((C, 1), "f")


# ---------------------------------------------------------------- program

def tap_conv(nc, x3, acc3, wcol, bcol, offs):
    """Depthwise conv via shifted tap accumulation, SAME zero padding.
    x3/acc3: [P, d1, d2] views.  offs: [(tap_idx, o1, o2)]; first = center."""
    d1, d2 = x3.shape[1], x3.shape[2]
    first = True
    for (j, o1, o2) in offs:
        lo1, hi1 = max(0, -o1), d1 - max(0, o1)
        lo2, hi2 = max(0, -o2), d2 - max(0, o2)
        src = x3[:, lo1 + o1:hi1 + o1, lo2 + o2:hi2 + o2]
        dst = acc3[:, lo1:hi1, lo2:hi2]
        if first:
            assert o1 == 0 and o2 == 0
            nc.vector.tensor_scalar(acc3[:, :, :], x3[:, :, :],
                                    wcol[:, j:j + 1], bcol[:, 0:1],
                                    AX.mult, AX.add)
            first = False
        else:
            nc.vector.scalar_tensor_tensor(dst, src, wcol[:, j:j + 1],
                                           dst, AX.mult, AX.add)


def build_vss_chunk(nc, tc, W, x_sb, out_d, ci, dil, main, scr, scanp,
                    scan1, psum, psy):
    r0 = 32 * ci
    ident = W["ident"]
    xi0 = scr.tile([C, L], BF16, tag="lnb")
    nc.scalar.activation(xi0[:], x_sb[r0:r0 + 32, :], ACTF.Copy)
    xi = xi0[:]
    xi3 = xi.rearrange("p (a b) -> p a b", a=HH)

    # ---- Stage A: axial depthwise convs + skip -> t_i [32, L]
    acc1 = scr.tile([C, L], F32, tag="S1")
    acc2 = scr.tile([C, L], F32, tag="S2")
    t_i = main.tile([C, L], F32, tag="t_i")
    a13 = acc1[:].rearrange("p (a b) -> p a b", a=HH)
    a23 = acc2[:].rearrange("p (a b) -> p a b", a=HH)
    offs = [(3, 0, 0)] + [(j, 0, (j - 3) * dil) for j in range(7) if j != 3]
    tap_conv(nc, xi3, a13, W[f"mwW{ci}"], W[f"mwB{ci}"], offs)
    offs = [(3, 0, 0)] + [(j, (j - 3) * dil, 0) for j in range(7) if j != 3]
    tap_conv(nc, a13, a23, W[f"mhW{ci}"], W[f"mhB{ci}"], offs)
    offs = [(4, 0, 0)] + [(3 * (dh + 1) + (dw + 1), dh * dil, dw * dil)
                          for dh in (-1, 0, 1) for dw in (-1, 0, 1)
                          if not (dh == 0 and dw == 0)]
    ti3 = t_i[:].rearrange("p (a b) -> p a b", a=HH)
    tap_conv(nc, a23, ti3, W[f"ccW{ci}"], W[f"ccB{ci}"], offs)
    nc.vector.tensor_tensor(t_i[:], t_i[:], xi, AX.add)

    # ---- Stage B: LayerNorm over c (t-partition layout)
    tT = scr.tile([128, 1024], BF16, tag="tT")    # [t128, (blk32, c32)]
    for half in range(2):
        ps = psum.tile([128, 512], F32, tag="ps")
        for b2 in range(16):
            blk = half * 16 + b2
            nc.tensor.transpose(ps[:, 32 * b2:32 * b2 + 32],
                                t_i[:, 128 * blk:128 * blk + 128],
                                ident[0:32, 0:32])
        nc.scalar.activation(tT[:, 512 * half:512 * half + 512], ps[:],
                             ACTF.Copy)
    t3 = tT[:].rearrange("p (a b) -> p a b", a=32)
    mu = scr.tile([128, 32], F32, tag="mu")
    nc.vector.tensor_reduce(mu[:, :, None], t3, mybir.AxisListType.X, AX.add)
    nc.vector.tensor_scalar_mul(mu[:], mu[:], -1.0 / 32)
    nc.vector.tensor_tensor(t3, t3,
                            mu[:, :, None].broadcast_to((128, 32, 32)),
                            AX.add)
    sq = scr.tile([128, 1024], BF16, tag="sq")
    sq3 = sq[:].rearrange("p (a b) -> p a b", a=32)
    nc.scalar.square(sq[:], tT[:])
    var = scr.tile([128, 32], F32, tag="var")
    nc.vector.tensor_reduce(var[:, :, None], sq3, mybir.AxisListType.X,
                            AX.add)
    nc.scalar.activation(var[:], var[:], ACTF.Ln,
                         bias=W["eps_col"][:, 0:1], scale=1.0 / 32)
    nc.scalar.activation(var[:], var[:], ACTF.Exp, scale=-0.5)
    nc.vector.tensor_tensor(t3, t3,
                            var[:, :, None].broadcast_to((128, 32, 32)),
                            AX.mult)
    # transpose back -> ln [32, L] bf16
    ln = scr.tile([C, L], BF16, tag="lnb")
    for g in range(8):
        ps = psy.tile([128, 512], BF16, tag="ypsum")
        for b2 in range(4):
            blk = g * 4 + b2
            nc.tensor.transpose(ps[0:32, 128 * b2:128 * b2 + 128],
                                tT[:, 32 * blk:32 * blk + 32],
                                W["identB"][:, :])
        nc.scalar.activation(ln[:, 512 * g:512 * g + 512], ps[0:32, :],
                             ACTF.Copy)

    # ---- Stage C: in_proj -> xcp (padded bf16 image), sz = silu(z+zb)
    # xcp layout: [64, (HH+2) rows x PW cols] zero-padded 3x3 halo
    xcp = scr.tile([DI, (HH + 2) * PW], BF16, tag="S1")
    nc.gpsimd.memset(xcp[:], 0.0)
    xcp3 = xcp[:].rearrange("p (a b) -> p a b", a=HH + 2)
    sz = main.tile([DI, L], BF16, tag="sz")
    for ch in range(8):
        ps = psum.tile([128, 512], F32, tag="ps")
        nc.tensor.matmul(ps[:], W["w_inT"][:], ln[:, 512 * ch:512 * ch + 512])
        # 512 cols = 8 image rows of 64; write into padded rows at (1,1)
        nc.scalar.activation(
            xcp3[:, 1 + 8 * ch:1 + 8 * ch + 8, 1:1 + WW],
            ps[0:64, :].rearrange("p (a b) -> p a b", a=8), ACTF.Copy)
        nc.scalar.activation(sz[:, 512 * ch:512 * ch + 512], ps[64:128, :],
                             ACTF.Silu, bias=W["z_bias"][:, 0:1])

    # ---- Stage D: 3x3 depthwise conv (PE diag taps) + silu -> u tiles
    u_t = main.tile([128, L], BF16, tag="u_t")   # rows: hw | wh
    u_wh = main.tile([DI, L], BF16, tag="u_wh")  # base-0 copy for PE rhs
    for hg in range(8):
        pd = psum.tile([128, 512], F32, tag="ps")
        for j, (dh, dw) in enumerate((a, b) for a in range(3)
                                     for b in range(3)):
            rhs = xcp3[:, 8 * hg + dh:8 * hg + dh + 8, dw:dw + WW]
            nc.tensor.matmul(pd[0:64, :], W["convDiag"][:, 64 * j:64 * j + 64],
                             rhs, start=(j == 0), stop=(j == 8))
        nc.scalar.activation(u_t[0:64, 512 * hg:512 * hg + 512], pd[0:64, :],
                             ACTF.Silu, bias=W["convB"][:, 0:1])
    u3 = u_t[0:64, :].rearrange("p (a b) -> p a b", a=HH)
    nc.vector.tensor_copy(u_wh[:].rearrange("p (a b) -> p a b", a=WW),
                          u3.transpose([0, 2, 1]))
    nc.vector.tensor_copy(u_t[64:128, :], u_wh[:])

    # ---- Stages E-F per k-pair tile
    oy = main.tile([DI, L], BF16, tag="oy")      # hw half (k0+k2)
    oyW = main.tile([DI, L], BF16, tag="oyW")    # wh half (k1+k3)
    for t in range(2):
        dt_t = scr.tile([128, L], BF16, tag="dt_t")
        w_t = scr.tile([128, L], BF16, tag="w_t")
        bbc = scr.tile([64, L], BF16, tag="bbc")
        for ch in range(8):
            sl = slice(512 * ch, 512 * ch + 512)
            ps = psum.tile([128, 512], F32, tag="ps")
            nc.tensor.matmul(ps[:], W[f"wdtBD{t}"][:], u_t[:, sl])
            sps = scr.tile([128, 512], BF16, tag="sps")
            nc.scalar.activation(sps[:], ps[:], ACTF.Exp,
                                 bias=W[f"dtb_col{t}"][:, 0:1])
            nc.scalar.activation(dt_t[:, sl], sps[:], ACTF.Ln, bias=1.0)
        for ch in range(8):
            sl = slice(512 * ch, 512 * ch + 512)
            ps = psum.tile([128, 512], F32, tag="ps")
            nc.tensor.matmul(ps[0:64, :], W[f"xprojBD{t}"][:], u_t[:, sl])
            nc.scalar.activation(bbc[:, sl], ps[0:64, :], ACTF.Copy)
        nc.vector.tensor_tensor(w_t[:], dt_t[:], u_t[:], AX.mult)

        flip = (t == 1)
        carry = scr.tile([128, N], F32, tag="carry")
        lcs = list(range(NCH)) if not flip else list(reversed(range(NCH)))
        for ilc, lc in enumerate(lcs):
            sl = slice(LC * lc, LC * lc + LC)
            yps = [psy.tile([128, 512], F32, tag="ypsum", name=f"yp{j2}")
                   for j2 in range(LC // 512)]
            for n in range(N):
                a_n = scanp.tile([128, LC], BF16, tag="a_n")
                nc.scalar.activation(a_n[:], dt_t[:, sl], ACTF.Exp,
                                     scale=W[f"A_cols{t}"][:, n:n + 1])
                Bb = scanp.tile([128, LC], BF16, tag="Bb")
                for j2 in range(LC // 512):
                    s3 = slice(LC * lc + 512 * j2, LC * lc + 512 * j2 + 512)
                    psb = psum.tile([128, 512], F32, tag="ps")
                    nc.tensor.matmul(psb[:], W[f"selnB{n}"][:], bbc[:, s3])
                    nc.scalar.activation(Bb[:, 512 * j2:512 * j2 + 512],
                                         psb[:], ACTF.Copy)
                b_n = scanp.tile([128, LC], BF16, tag="b_n")
                nc.vector.tensor_tensor(b_n[:], w_t[:, sl], Bb[:], AX.mult)
                h_n = scan1.tile([128, LC], BF16, tag="h_n")
                init = 0.0 if ilc == 0 else carry[:, n:n + 1]
                if not flip:
                    nc.vector.tensor_tensor_scan(h_n[:], a_n[:], b_n[:],
                                                 init, AX.mult, AX.add)
                    if ilc == 0 and NCH > 1:
                        nc.scalar.activation(carry[:, n:n + 1],
                                             h_n[:, LC - 1:LC], ACTF.Copy)
                else:
                    nc.vector.tensor_tensor_scan(_flip(h_n[:]), _flip(a_n[:]),
                                                 _flip(b_n[:]), init,
                                                 AX.mult, AX.add)
                    if ilc == 0 and NCH > 1:
                        nc.scalar.activation(carry[:, n:n + 1],
                                             h_n[:, 0:1], ACTF.Copy)
                Cb = scanp.tile([128, LC], BF16, tag="Cb")
                for j2 in range(LC // 512):
                    s3 = slice(LC * lc + 512 * j2, LC * lc + 512 * j2 + 512)
                    psc = psum.tile([128, 512], F32, tag="ps")
                    nc.tensor.matmul(psc[:], W[f"selnC{n}"][:], bbc[:, s3])
                    nc.scalar.activation(Cb[:, 512 * j2:512 * j2 + 512],
                                         psc[:], ACTF.Copy)
                hc_n = scanp.tile([128, LC], BF16, tag="hc_n")
                nc.gpsimd.tensor_tensor(hc_n[:], h_n[:], Cb[:], AX.mult)
                for j in range(LC // 512):
                    nc.tensor.matmul(yps[j][:], W["identB"][:, :],
                                     hc_n[:, 512 * j:512 * j + 512],
                                     start=(n == 0), stop=False)
            for j in range(LC // 512):
                s2 = slice(LC * lc + 512 * j, LC * lc + 512 * j + 512)
                nc.tensor.matmul(yps[j][:], W[f"DdiagL{t}"][:],
                                 u_t[0:64, s2], start=False, stop=False)
                nc.tensor.matmul(yps[j][:], W[f"DdiagH{t}"][:], u_wh[:, s2],
                                 start=False, stop=True)
                if t == 0:
                    nc.vector.tensor_copy(oy[:, s2], yps[j][0:64, :])
                    nc.vector.tensor_copy(oyW[:, s2], yps[j][64:128, :])
                else:
                    nc.vector.tensor_tensor(oy[:, s2], oy[:, s2],
                                            yps[j][0:64, :], AX.add)
                    nc.vector.tensor_tensor(oyW[:, s2], oyW[:, s2],
                                            yps[j][64:128, :], AX.add)

    # ---- Stage G: combine directions -> y_d [64, L] bf16
    y_d = scr.tile([DI, L], BF16, tag="lnb")
    oy_wh = oyW[:].rearrange("p (a b) -> p a b", a=WW)
    y3 = y_d[:].rearrange("p (a b) -> p a b", a=HH)
    nc.vector.tensor_tensor(
        y3, oy[:].rearrange("p (a b) -> p a b", a=HH),
        oy_wh.transpose([0, 2, 1]), AX.add)

    # ---- Stage H: out-LN over d, gate with silu(z), out_proj, residual
    yT = scr.tile([128, 2048], F32, tag="S2")    # [t128, (blk32, d64)]
    for g in range(4):
        ps = psy.tile([128, 512], BF16, tag="ypsum")
        for b2 in range(8):
            blk = g * 8 + b2
            nc.tensor.transpose(ps[:, 64 * b2:64 * b2 + 64],
                                y_d[:, 128 * blk:128 * blk + 128],
                                W["identB"][0:64, 0:64])
        nc.scalar.activation(yT[:, 512 * g:512 * g + 512], ps[:], ACTF.Copy)
    y3T = yT[:].rearrange("p (a b) -> p a b", a=32)
    mu2 = scr.tile([128, 32], F32, tag="mu")
    nc.vector.tensor_reduce(mu2[:, :, None], y3T, mybir.AxisListType.X,
                            AX.add)
    nc.vector.tensor_scalar_mul(mu2[:], mu2[:], -1.0 / 64)
    nc.vector.tensor_tensor(y3T, y3T,
                            mu2[:, :, None].broadcast_to((128, 32, 64)),
                            AX.add)
    var2 = scr.tile([128, 32], F32, tag="var")
    for hf in range(2):
        sq2 = scr.tile([128, 1024], BF16, tag="sq")
        sq23 = sq2[:].rearrange("p (a b) -> p a b", a=16)
        y3Th = yT[:, 1024 * hf:1024 * hf + 1024].rearrange(
            "p (a b) -> p a b", a=16)
        nc.scalar.square(sq2[:], yT[:, 1024 * hf:1024 * hf + 1024])
        nc.vector.tensor_reduce(var2[:, 16 * hf:16 * hf + 16, None], sq23,
                                mybir.AxisListType.X, AX.add)
    nc.scalar.activation(var2[:], var2[:], ACTF.Ln,
                         bias=W["eps_col"][:, 0:1], scale=1.0 / 64)
    nc.scalar.activation(var2[:], var2[:], ACTF.Exp, scale=-0.5)
    nc.vector.tensor_tensor(y3T, y3T,
                            var2[:, :, None].broadcast_to((128, 32, 64)),
                            AX.mult)
    # gate with silu(z) (transposed via PE)
    for g in range(4):
        ps = psy.tile([128, 512], BF16, tag="ypsum")
        for b2 in range(8):
            blk = g * 8 + b2
            nc.tensor.transpose(ps[:, 64 * b2:64 * b2 + 64],
                                sz[:, 128 * blk:128 * blk + 128],
                                W["identB"][0:64, 0:64])
        nc.vector.tensor_tensor(yT[:, 512 * g:512 * g + 512],
                                yT[:, 512 * g:512 * g + 512], ps[:], AX.mult)
    # transpose back -> gy [64, L] bf16
    gy = scr.tile([DI, L], BF16, tag="S1")
    for g in range(8):
        ps = psum.tile([128, 512], F32, tag="ps")
        for b2 in range(4):
            blk = g * 4 + b2
            nc.tensor.transpose(ps[0:64, 128 * b2:128 * b2 + 128],
                                yT[:, 64 * blk:64 * blk + 64], ident[:, :])
        nc.scalar.activation(gy[:, 512 * g:512 * g + 512], ps[0:64, :],
                             ACTF.Copy)
    # out_proj + residual (in-place into t_i) + BN + ReLU -> DRAM rows
    for ch in range(8):
        ps = psum.tile([128, 512], F32, tag="ps")
        nc.tensor.matmul(ps[0:32, :], W["w_outT"][:],
                         gy[:, 512 * ch:512 * ch + 512])
        nc.vector.tensor_tensor(t_i[:, 512 * ch:512 * ch + 512],
                                t_i[:, 512 * ch:512 * ch + 512],
                                ps[0:32, :], AX.add)
    nc.scalar.activation(t_i[:], t_i[:], ACTF.Relu,
                         bias=W["bn_bias"][r0:r0 + 32, 0:1],
                         scale=W["bn_s"][r0:r0 + 32, 0:1])
    nc.sync.dma_start(out_d[r0:r0 + 32, :], t_i[:])


def build_program():
    nc = bacc.Bacc("TRN2", debug=False, num_devices=8)
    xs_d = nc.dram_tensor("xs", [128, L], F32, kind="ExternalInput")
    wts = {name: nc.dram_tensor(name, list(shape),
                                F32 if dt_ == "f" else BF16,
                                kind="ExternalInput")
           for name, (shape, dt_) in WSPEC.items()}
    out_d = nc.dram_tensor("out", [128, L], F32, kind="ExternalOutput")

    with tile.TileContext(nc) as tc:
        with contextlib.ExitStack() as ctx:
            cpool = ctx.enter_context(tc.tile_pool(name="consts", bufs=1))
            main = ctx.enter_context(tc.tile_pool(name="main", bufs=1))
            scr = ctx.enter_context(tc.tile_pool(name="scr", bufs=1))
            scanp = ctx.enter_context(tc.tile_pool(name="scanp", bufs=2))
            scan1 = ctx.enter_context(tc.tile_pool(name="scan1", bufs=2))
            psum = ctx.enter_context(
                tc.tile_pool(name="ps", bufs=4, space="PSUM"))
            psy = ctx.enter_context(
                tc.tile_pool(name="psy", bufs=4, space="PSUM"))

            W = {}
            for name, (shape, dt_) in WSPEC.items():
                t = cpool.tile(list(shape), F32 if dt_ == "f" else BF16,
                               tag=f"w_{name}")
                nc.sync.dma_start(t[:], wts[name][:])
                W[name] = t

            x_sb = main.tile([128, L], F32, tag="x_sb")
            nc.sync.dma_start(x_sb[:], xs_d[:])

            for ci in range(3):
                build_vss_chunk(nc, tc, W, x_sb, out_d, ci, ci + 1,
                                main, scr, scanp, scan1, psum, psy)

            # chunk 4 passes through: BN + ReLU only
            x4o = scr.tile([C, L], F32, tag="S2")
            nc.scalar.activation(x4o[:], x_sb[96:128, :], ACTF.Relu,
                                 bias=W["bn_bias"][96:128, 0:1],
                                 scale=W["bn_s"][96:128, 0:1])
            nc.sync.dma_start(out_d[96:128, :], x4o[:])

    nc.compile()
    return nc


def get_program():
    if "nc" not in _CACHE:
        _CACHE["nc"] = build_program()
    return _CACHE["nc"]


def kernel(**inputs):
    import os
    from concourse.bass_utils import run_bass_kernel_spmd
    global LAST_EXEC_NS
    nc = get_program()
    w = build_weights(inputs)
    x = np.asarray(inputs["x"], np.float32)          # [8, 128, 64, 64]
    in_maps = []
    for core in range(8):
        m = dict(w)
        m["xs"] = np.ascontiguousarray(x[core].reshape(128, L))
        in_maps.append(m)
    kw = {}
    if os.environ.get("BASS_PROF_DIR"):
        kw = dict(tmpdir=os.environ["BASS_PROF_DIR"], trace=True)
    res = run_bass_kernel_spmd(nc, in_maps, list(range(8)), **kw)
    LAST_EXEC_NS = res.exec_time_ns
    out = np.stack([res.results[i]["out"] for i in range(8)])
    return out.reshape(8, 128, HH, WW).astype(np.float32)
